# revision 1
# baseline (speedup 1.0000x reference)
"""Cosformer attention Trainium2 kernel.

Shards batch*heads across 8 NeuronCores: core c handles batch c//4 and
heads 4*(c%4) .. 4*(c%4)+4 (a 256-wide slice of the embedding). Each core:
  - projects q/k/v from its batch's query slice (fp32r matmuls),
  - applies RoPE + relu,
  - computes the per-head cosformer kv summary (2D x D) + k-sums over all L,
  - applies q' @ kv with the 1/max(denom,eps) normalization,
  - multiplies by its slice of Wo, producing a partial (L, E) output.
Host sums the 4 partials per batch. No cross-device communication.
"""

import os
import sys

if "/opt/trn_rl_repo" not in sys.path:
    sys.path.insert(0, "/opt/trn_rl_repo")

from contextlib import ExitStack

import numpy as np

import concourse.bass as bass
import concourse.bacc as bacc
import concourse.mybir as mybir
import concourse.tile as tile
from concourse.masks import make_identity

F32 = mybir.dt.float32
F32R = mybir.dt.float32r
EPS = 1e-6

L_FULL, N_BATCH, E, H, D = 4096, 2, 1024, 16, 64
N_CORES = 8
HEADS_PER_CORE = 4          # 2 pairs
F_LOC = HEADS_PER_CORE * D  # 256


def build_program(LT=32, stage="J"):
    """Build the single-core SPMD Bass program. LT = number of 128-row L tiles."""
    L = LT * 128
    NCH = max(LT // 4, 1)       # 512-row DMA chunks
    LT_PER_CH = LT // NCH       # 4 (or LT when LT < 4)
    CH = LT_PER_CH * 128        # 512

    nc = bacc.Bacc("TRN2", target_bir_lowering=False, debug=False)

    qbT_d = nc.dram_tensor("qbT", [E, L], F32R, kind="ExternalInput").ap()
    wqT_d = nc.dram_tensor("wqT", [E, F_LOC], F32R, kind="ExternalInput").ap()
    wkvT_d = nc.dram_tensor("wkvT", [E, 2 * F_LOC], F32R, kind="ExternalInput").ap()
    wo_d = nc.dram_tensor("wo_rhs", [F_LOC, E], F32R, kind="ExternalInput").ap()
    cosT_d = nc.dram_tensor("cosT_rep", [128, L], F32, kind="ExternalInput").ap()
    sinT_d = nc.dram_tensor("sinT_rep", [128, L], F32, kind="ExternalInput").ap()
    cosL_d = nc.dram_tensor("cosL", [128, LT * D], F32, kind="ExternalInput").ap()
    sinmL_d = nc.dram_tensor("sinmL", [128, LT * D], F32, kind="ExternalInput").ap()
    sc_d = nc.dram_tensor("sc_t", [128, LT * 2], F32, kind="ExternalInput").ap()
    scr_d = nc.dram_tensor("sc_tr", [128, LT * 2], F32R, kind="ExternalInput").ap()
    p2_d = nc.dram_tensor("p2_rot", [128, 128], F32R, kind="ExternalInput").ap()
    id_d = nc.dram_tensor("ident_r", [128, 128], F32R, kind="ExternalInput").ap()
    out_d = nc.dram_tensor("out_partial", [L, E], F32, kind="ExternalOutput").ap()

    with tile.TileContext(nc) as tc, ExitStack() as ctx:
        consts = ctx.enter_context(tc.tile_pool(name="consts", bufs=1))

        # ---- resident constants ----
        wq_sb = consts.tile([128, 8 * F_LOC], F32R, name="wq_sb")
        wkv_sb = consts.tile([128, 8 * 2 * F_LOC], F32R, name="wkv_sb")
        wo_sb = consts.tile([128, 2 * E], F32R, name="wo_sb")
        cosT_sb = consts.tile([128, L], F32, name="cosT_sb")
        sinT_sb = consts.tile([128, L], F32, name="sinT_sb")
        cosL_sb = consts.tile([128, LT * D], F32, name="cosL_sb")
        sinmL_sb = consts.tile([128, LT * D], F32, name="sinmL_sb")
        sc_sb = consts.tile([128, LT * 2], F32, name="sc_sb")
        scr_sb = consts.tile([128, LT * 2], F32R, name="scr_sb")
        p2_sb = consts.tile([128, 128], F32R, name="p2_sb")
        ident = consts.tile([128, 128], F32R, name="ident")

        def load_small_consts():
            nc.sync.dma_start(sc_sb[:], sc_d[:])
            nc.sync.dma_start(scr_sb[:], scr_d[:])
            nc.sync.dma_start(p2_sb[:], p2_d[:])
            nc.sync.dma_start(ident[:], id_d[:])
            nc.sync.dma_start(cosL_sb[:], cosL_d[:])
            nc.sync.dma_start(sinmL_sb[:], sinmL_d[:])

        def load_trig_consts():
            nc.sync.dma_start(cosT_sb[:], cosT_d[:])
            nc.sync.dma_start(sinT_sb[:], sinT_d[:])

        def load_wo():
            nc.sync.dma_start(
                wo_sb[:].rearrange("p (k j) -> p k j", k=2),
                wo_d.rearrange("(k p) j -> p k j", p=128),
            )

        # resident q^T (f-major, per pair) and kv summaries
        qT_sb = consts.tile([128, 2 * L], F32R, name="qT_sb")
        qT3 = qT_sb[:].rearrange("p (r l) -> p r l", r=2)
        kv_sb = [consts.tile([128, 260], F32R, name=f"kv_sb{pr}") for pr in range(2)]
        for pr in range(2):
            nc.vector.memset(kv_sb[pr][:].bitcast(F32), 0.0)

        wq3 = wq_sb[:].rearrange("p (e f) -> p e f", e=8)
        wkv3 = wkv_sb[:].rearrange("p (e f) -> p e f", e=8)
        wo3 = wo_sb[:].rearrange("p (k j) -> p k j", k=2)

        # ================= PASS 1 =================
        with ExitStack() as p1:
            qch_pool = p1.enter_context(tc.tile_pool(name="qch", bufs=3))
            proj_ps = p1.enter_context(tc.tile_pool(name="proj_ps", bufs=3, space="PSUM"))
            q_ps_pool = p1.enter_context(tc.tile_pool(name="q_ps", bufs=3, space="PSUM"))
            kv_ps_pool = p1.enter_context(tc.tile_pool(name="kv_ps", bufs=1, space="PSUM"))
            wk1 = p1.enter_context(tc.tile_pool(name="wk1", bufs=3))

            kv_ps = [kv_ps_pool.tile([128, 258], F32, name=f"kv_ps{pr}") for pr in range(2)]

            wq3_ = wq_sb[:].rearrange("p (e f) -> p e f", e=8)
            wkv3_ = wkv_sb[:].rearrange("p (e f) -> p e f", e=8)
            wqd3 = wqT_d.rearrange("(e p) f -> p e f", p=128)
            wkvd3 = wkvT_d.rearrange("(e p) f -> p e f", p=128)
            qd3_full = qbT_d.rearrange("(e p) l -> p e l", p=128)
            load_small_consts()
            qc_first = qch_pool.tile([128, 8 * CH], F32R, name="qc", tag="qc")
            qcf3 = qc_first[:].rearrange("p (e l) -> p e l", e=8)
            for e in range(8):
                nc.sync.dma_start(qcf3[:, e:e + 1, :], qd3_full[:, e:e + 1, 0:CH])
                nc.sync.dma_start(wq3_[:, e:e + 1, :], wqd3[:, e:e + 1, :])
                nc.sync.dma_start(wkv3_[:, e:e + 1, :], wkvd3[:, e:e + 1, :])
            load_trig_consts()

            for ch in range(NCH):
                if ch == 0:
                    qc = qc_first
                else:
                    qc = qch_pool.tile([128, 8 * CH], F32R, name="qc", tag="qc")
                qc3 = qc[:].rearrange("p (e l) -> p e l", e=8)
                if ch > 0:
                    qd3 = qd3_full[:, :, ch * CH:(ch + 1) * CH]
                    for e in range(8):
                        nc.sync.dma_start(qc3[:, e:e + 1, :], qd3[:, e:e + 1, :])
                if ch == min(1, NCH - 1):
                    load_wo()

                # ---- q: f-major projection + relu + RoPE (per pair) ----
                for pr in range(2 if stage >= "C" else 0):
                    qT_ps = q_ps_pool.tile([128, CH], F32, name="qT_ps", tag="q512")
                    for e in range(8):
                        nc.tensor.matmul(
                            qT_ps[:],
                            wq3[:, e, pr * 128:(pr + 1) * 128],
                            qc3[:, e, :],
                            start=(e == 0), stop=(e == 7),
                        )
                    qTr = wk1.tile([128, CH], F32R, name="qTr", tag="qTr")
                    nc.scalar.activation(qTr[:], qT_ps[:], mybir.ActivationFunctionType.Relu)
                    rot_ps = q_ps_pool.tile([128, CH], F32, name="rot_ps", tag="q512")
                    nc.tensor.matmul(
                        rot_ps[:], p2_sb[:], qTr[:],
                        start=True, stop=True,
                    )
                    csl = slice(ch * CH, (ch + 1) * CH)
                    tq = wk1.tile([128, CH], F32, name="tq", tag="tq")
                    nc.vector.tensor_tensor(
                        out=tq[:], in0=qTr[:], in1=cosT_sb[:, csl], op=mybir.AluOpType.mult)
                    uq = wk1.tile([128, CH], F32, name="uq", tag="uq")
                    nc.vector.tensor_tensor(
                        out=uq[:], in0=rot_ps[:], in1=sinT_sb[:, csl], op=mybir.AluOpType.mult)
                    nc.vector.tensor_tensor(
                        out=qT3[:, pr, csl], in0=tq[:], in1=uq[:], op=mybir.AluOpType.add)

                # ---- k, v: L-major projections, per L-tile ----
                for lt in range(LT_PER_CH if stage >= "D" else 0):
                    t = ch * LT_PER_CH + lt
                    lsl = slice(lt * 128, (lt + 1) * 128)
                    kv_proj_ps = proj_ps.tile([128, 2 * F_LOC], F32, name="kv_proj_ps", tag="proj")
                    k_ps = kv_proj_ps[:, 0:F_LOC]
                    v_ps = kv_proj_ps[:, F_LOC:2 * F_LOC]
                    for e in range(8):
                        nc.tensor.matmul(kv_proj_ps[:], qc3[:, e, lsl], wkv3[:, e, :],
                                         start=(e == 0), stop=(e == 7))

                    # k: relu then RoPE (L-major; swap via reversed AP, signed sin)
                    k_sb = wk1.tile([128, F_LOC], F32, name="k_sb", tag="k_sb")
                    nc.scalar.activation(k_sb[:], k_ps, mybir.ActivationFunctionType.Relu)
                    dsl = slice(t * D, (t + 1) * D)
                    cosL_t = cosL_sb[:, dsl].rearrange("p (a j) -> p a j", a=2) \
                        .unsqueeze(1).broadcast_to([128, 4, 2, 32])
                    sinm_t = sinmL_sb[:, dsl].rearrange("p (a j) -> p a j", a=2) \
                        .unsqueeze(1).broadcast_to([128, 4, 2, 32])
                    kt1 = wk1.tile([128, F_LOC], F32R, name="kt1", tag="kt1")
                    nc.vector.tensor_tensor(
                        out=kt1[:].rearrange("p (h a j) -> p h a j", h=4, a=2),
                        in0=k_sb[:].rearrange("p (h a j) -> p h a j", h=4, a=2),
                        in1=cosL_t, op=mybir.AluOpType.mult)
                    kt2 = wk1.tile([128, F_LOC], F32, name="kt2", tag="kt2")
                    nc.vector.tensor_tensor(
                        out=kt2[:].rearrange("p (h a j) -> p h a j", h=4, a=2),
                        in0=k_sb[:].rearrange("p (h a j) -> p h a j", h=4, a=2)[:, :, ::-1, :],
                        in1=sinm_t, op=mybir.AluOpType.mult)
                    nc.vector.tensor_tensor(out=kt1[:], in0=kt1[:], in1=kt2[:],
                                            op=mybir.AluOpType.add)

                    # v: per-pair scaled evictions [vs|vc|s|c] x 2
                    vsc = wk1.tile([128, 520], F32R, name="vsc", tag="vsc")
                    s_col = sc_sb[:, 2 * t:2 * t + 1]
                    c_col = sc_sb[:, 2 * t + 1:2 * t + 2]
                    for pr in range(2):
                        base = pr * 260
                        vp = kv_proj_ps[:, F_LOC + pr * 128:F_LOC + (pr + 1) * 128]
                        nc.scalar.activation(vsc[:, base:base + 128], vp,
                                             mybir.ActivationFunctionType.Copy, scale=s_col)
                        nc.scalar.activation(vsc[:, base + 128:base + 256], vp,
                                             mybir.ActivationFunctionType.Copy, scale=c_col)
                        nc.vector.tensor_copy(vsc[:, base + 256:base + 258],
                                              scr_sb[:, 2 * t:2 * t + 2])

                    # kv accumulation (single group per pair incl k-sums)
                    for pr in range(2 if stage >= "E" else 0):
                        psl = slice(pr * 128, (pr + 1) * 128)
                        nc.tensor.matmul(
                            kv_ps[pr][:],
                            kt1[:, psl],
                            vsc[:, pr * 260:pr * 260 + 258],
                            start=(t == 0), stop=(t == LT - 1),
                        )

            # ---- kv eviction / per-head rearrangement ----
            # kv_ps[pr]: rows 0:64 = head A (d), 64:128 = head B;
            # cols 0:128 kv_top (A cols 0:64, B cols 64:128), 128:256 kv_bot, 256:258 ksum t/b.
            # kv_sb[pr]: cols 0:64 own kv_top, 64 ksum_top, 65:129 own kv_bot, 129 ksum_bot.
            # kv_sb[pr] block-diagonal (128, 260): rows 0:64 (head A d) hold A's
            # [kvtop|kstop|kvbot|ksbot] in cols 0:130; rows 64:128 hold B's in cols 130:260.
            for pr in range(2 if stage >= "F" else 0):
                nc.vector.tensor_copy(kv_sb[pr][0:64, 0:64], kv_ps[pr][0:64, 0:64])
                nc.vector.tensor_copy(kv_sb[pr][0:64, 64:65], kv_ps[pr][0:64, 256:257])
                nc.vector.tensor_copy(kv_sb[pr][0:64, 65:129], kv_ps[pr][0:64, 128:192])
                nc.vector.tensor_copy(kv_sb[pr][0:64, 129:130], kv_ps[pr][0:64, 257:258])
                nc.vector.tensor_copy(kv_sb[pr][64:128, 130:194], kv_ps[pr][64:128, 64:128])
                nc.vector.tensor_copy(kv_sb[pr][64:128, 194:195], kv_ps[pr][64:128, 256:257])
                nc.vector.tensor_copy(kv_sb[pr][64:128, 195:259], kv_ps[pr][64:128, 192:256])
                nc.vector.tensor_copy(kv_sb[pr][64:128, 259:260], kv_ps[pr][64:128, 257:258])

        # ================= PASS 2 =================
        with ExitStack() as p2:
            ab_ps_pool = p2.enter_context(tc.tile_pool(name="ab_ps", bufs=3, space="PSUM"))
            tp_ps_pool = p2.enter_context(tc.tile_pool(name="tp_ps", bufs=2, space="PSUM"))
            op_ps_pool = p2.enter_context(tc.tile_pool(name="op_ps", bufs=3, space="PSUM"))
            wk2 = p2.enter_context(tc.tile_pool(name="wk2", bufs=3))

            _g = {"G0": (1, 1, 1), "G1": (1, 1, 2), "G2": (1, 2, 2), "G3": (LT, 2, 2)}
            _nt, _npr, _nhh = _g.get(stage, (LT, 2, 2))
            for t in range(_nt if stage >= "G" else 0):
                l0 = t * 128
                lsl = slice(l0, l0 + 128)
                s_col = sc_sb[:, 2 * t:2 * t + 1]
                c_col = sc_sb[:, 2 * t + 1:2 * t + 2]
                attnT = []
                for pr in range(_npr):
                    ab_ps = ab_ps_pool.tile([128, 260], F32, name="ab_ps", tag="ab")
                    nc.tensor.matmul(
                        ab_ps[:],
                        qT3[:, pr:pr + 1, lsl].squeeze(1),
                        kv_sb[pr][:],
                        start=True, stop=True,
                    )
                    ab3 = ab_ps[:].rearrange("p (h x) -> p h x", h=2)
                    if stage < "H":
                        continue
                    tmp = wk2.tile([128, 130], F32, name="tmp", tag="tmp")
                    tmp3 = tmp[:].rearrange("p (h x) -> p h x", h=2)
                    nc.scalar.activation(tmp3, ab3[:, :, 65:130],
                                         mybir.ActivationFunctionType.Copy, scale=c_col)
                    nd = wk2.tile([128, 130], F32, name="nd", tag="nd")
                    nd3 = nd[:].rearrange("p (h x) -> p h x", h=2)
                    nc.vector.scalar_tensor_tensor(
                        out=nd3, in0=ab3[:, :, 0:65], scalar=s_col, in1=tmp3,
                        op0=mybir.AluOpType.mult, op1=mybir.AluOpType.add)
                    z = wk2.tile([128, 2], F32, name="z", tag="z")
                    nc.vector.tensor_scalar_max(z[:], nd3[:, :, 64], EPS)
                    nc.vector.reciprocal(z[:], z[:])
                    attn = wk2.tile([128, 128], F32R, name="attn", tag="attn")
                    nc.vector.tensor_tensor(
                        out=attn[:].rearrange("p (h j) -> p h j", h=2),
                        in0=nd3[:, :, 0:64],
                        in1=z[:].unsqueeze(2).broadcast_to([128, 2, 64]),
                        op=mybir.AluOpType.mult)
                    if stage < "I":
                        continue
                    tp_ps = tp_ps_pool.tile([128, 128], F32R, name="tp_ps", tag="tp")
                    nc.tensor.transpose(tp_ps[:], attn[:], ident[:])
                    aT = wk2.tile([128, 128], F32R, name="aT", tag="aT", bufs=6)
                    nc.scalar.activation(aT[:], tp_ps[:], mybir.ActivationFunctionType.Copy)
                    attnT.append(aT)

                if stage < "J" or len(attnT) < 2:
                    continue
                out_sb = wk2.tile([128, E], F32, name="out_sb", tag="out_sb", bufs=3)
                for nck in range(2):
                    op_ps = op_ps_pool.tile([128, 512], F32, name="op_ps", tag="op")
                    for pr in range(2):
                        nc.tensor.matmul(
                            op_ps[:],
                            attnT[pr][:],
                            wo3[:, pr, nck * 512:(nck + 1) * 512],
                            start=(pr == 0), stop=(pr == 1),
                        )
                    if nck == 0:
                        nc.scalar.activation(out_sb[:, 0:512], op_ps[:],
                                             mybir.ActivationFunctionType.Copy)
                    else:
                        nc.vector.tensor_copy(out_sb[:, 512:1024], op_ps[:])
                nc.sync.dma_start(out_d[lsl, :], out_sb[:])

    nc.compile()
    return nc


def host_prep(query, cos, sin, Wq, Wk, Wv, Wo, L=L_FULL, LT=32):
    """Build per-core input maps."""
    cos2 = np.ascontiguousarray(cos[0], dtype=np.float32)   # (L, D)
    sin2 = np.ascontiguousarray(sin[0], dtype=np.float32)
    cosT_rep = np.ascontiguousarray(np.tile(cos2.T, (2, 1)), dtype=np.float32)  # (128, L)
    sinT_rep = np.ascontiguousarray(np.tile(sin2.T, (2, 1)), dtype=np.float32)
    cosL = np.ascontiguousarray(
        cos2.reshape(LT, 128, D).transpose(1, 0, 2).reshape(128, LT * D))
    sinm2 = np.concatenate([-sin2[:, :D // 2], sin2[:, D // 2:]], axis=1)
    sinmL = np.ascontiguousarray(
        sinm2.reshape(LT, 128, D).transpose(1, 0, 2).reshape(128, LT * D))

    idx = (np.pi / 2) * np.arange(1, L + 1, dtype=np.float32) / L
    s_arr = np.sin(idx).astype(np.float32)
    c_arr = np.cos(idx).astype(np.float32)
    sc = np.empty((128, LT * 2), dtype=np.float32)
    sc[:, 0::2] = s_arr.reshape(LT, 128).T
    sc[:, 1::2] = c_arr.reshape(LT, 128).T

    p_rot = np.zeros((D, D), dtype=np.float32)
    for j in range(D // 2):
        p_rot[D // 2 + j, j] = -1.0   # rot[:, j] = -q[:, 32+j]
        p_rot[j, D // 2 + j] = 1.0    # rot[:, 32+j] = q[:, j]
    p2 = np.zeros((128, 128), dtype=np.float32)
    p2[0:64, 0:64] = p_rot
    p2[64:128, 64:128] = p_rot

    qbT = [np.ascontiguousarray(query[:, b, :].T) for b in range(N_BATCH)]

    in_maps = []
    for c in range(N_CORES):
        b = c // 4
        r0 = (c % 4) * F_LOC
        in_maps.append({
            "qbT": qbT[b],
            "wqT": np.ascontiguousarray(Wq[r0:r0 + F_LOC, :].T),
            "wkvT": np.ascontiguousarray(
                np.concatenate([Wk[r0:r0 + F_LOC, :].T, Wv[r0:r0 + F_LOC, :].T], axis=1)),
            "wo_rhs": np.ascontiguousarray(Wo[:, r0:r0 + F_LOC].T),
            "cosT_rep": cosT_rep,
            "sinT_rep": sinT_rep,
            "cosL": cosL,
            "sinmL": sinmL,
            "sc_t": sc,
            "sc_tr": sc,
            "p2_rot": p2,
            "ident_r": np.eye(128, dtype=np.float32),
        })
    return in_maps


_PROG_CACHE = {}


def run(inputs, trace=False, trace_kwargs=None):
    """Run on 8 NeuronCores; returns (output, BassKernelResults)."""
    from concourse.bass_utils import run_bass_kernel_spmd

    LT = L_FULL // 128
    if LT not in _PROG_CACHE:
        _PROG_CACHE[LT] = build_program(LT)
    nc = _PROG_CACHE[LT]
    in_maps = host_prep(**inputs)
    kw = {}
    if trace:
        kw["trace"] = True
        if trace_kwargs:
            kw.update(trace_kwargs)
    res = run_bass_kernel_spmd(nc, in_maps, core_ids=list(range(N_CORES)), **kw)
    partials = [res.results[c]["out_partial"] for c in range(N_CORES)]
    out = np.empty((L_FULL, N_BATCH, E), dtype=np.float32)
    for b in range(N_BATCH):
        acc = partials[4 * b].copy()
        for c in range(4 * b + 1, 4 * b + 4):
            acc += partials[c]
        out[:, b, :] = acc
    return out, res


def kernel(**inputs):
    out, _ = run(inputs)
    return out

